# revision 28
# baseline (speedup 1.0000x reference)
# Bahdanau-style additive attention, fused single-pass Trainium2 kernel.
#
# Reference computation (per batch element b):
#   f_proj = features[b] @ W1_w + W1_b          [L, U]
#   h_proj = hidden[b] @ W2_w + W2_b            [U]
#   score  = tanh(f_proj + h_proj)              [L, U]
#   logits = score @ V_w (+ V_b)                [L]
#   aw     = softmax_L(logits)                  [L]   (V_b cancels in softmax)
#   ctx    = sum_l aw[l] * features[b, l, :]    [D]
#
# Sharding: data-parallel over batch B=64 across 8 NeuronCores (8 per core),
# weights replicated. Everything between the feature load and the two tiny
# outputs stays on-chip.
#
# Per-core structure (b = 0..7 local batches), fully software-pipelined:
#  - nat[b,j] = [128(l), 1024(d)] natural feature tiles, resident (reused by
#    the context matmul at the end of each batch).
#  - featT[b,k] = [128(d), 256(l)] via PE transposes + one DVE copy per tile.
#  - W1/W2 streamed as column blocks [H,128] -> [128,(k,u)] so u-tile m only
#    waits for its own 512KB, letting the GEMM start ~8us into the kernel.
#  - f_projT accumulated over 8 d-tiles in PSUM; ScalarE tanh(psum + bias)
#    with bias = (hidden@W2 + W1_b + W2_b)[u] per partition, batched upfront.
#  - logits row via matmul with V column stationary, accumulated over u-tiles.
#  - per-batch softmax on the [1,256] PSUM row (exp w/ fused row-sum on ACT),
#    aw transposed back to a [128,1] column, context row via matmul with the
#    aw column stationary against nat tiles; all overlapped with the next
#    batch's GEMM.
#
# Matmuls/transposes run as float32r (fp32 replication mode): measured
# ~1.3 cyc/row on HW vs 5 cyc/row for plain fp32, at ~2e-4 relative error.
# Walrus requires every f32r matmul operand to be *produced* as f32r, so the
# feeding tiles are declared float32r; non-matmul readers bitcast to f32.

import os
from contextlib import ExitStack

import numpy as np

import concourse.mybir as mybir
import concourse.tile as tile
from concourse import bacc
from concourse.bass_utils import run_bass_kernel_spmd

B, L, D, H, U = 64, 256, 1024, 1024, 1024
NCORES = 8
BPC = B // NCORES  # local batches per core
KL = L // 128      # 2 l-tiles
KD = D // 128      # 8 d-tiles
KU = U // 128      # 8 u-tiles
KH = H // 128      # 8 h-tiles

F32 = mybir.dt.float32
F32R = mybir.dt.float32r
AF = mybir.ActivationFunctionType


def _r(ap):
    return ap.bitcast(F32R)


def _f(ap):
    return ap.bitcast(F32)


def build_nc(pipeline_reps=1):
    nc = bacc.Bacc(target_bir_lowering=False, debug=False)

    feat = nc.dram_tensor("features", [BPC, L, D], F32, kind="ExternalInput").ap()
    hid = nc.dram_tensor("hidden", [BPC, H], F32, kind="ExternalInput").ap()
    w1 = nc.dram_tensor("W1_w", [D, U], F32, kind="ExternalInput").ap()
    w1b = nc.dram_tensor("W1_b", [U], F32, kind="ExternalInput").ap()
    w2 = nc.dram_tensor("W2_w", [H, U], F32, kind="ExternalInput").ap()
    w2b = nc.dram_tensor("W2_b", [U], F32, kind="ExternalInput").ap()
    vw = nc.dram_tensor("V_w", [U, 1], F32, kind="ExternalInput").ap()
    ctx_out = nc.dram_tensor("ctx_out", [BPC, D], F32, kind="ExternalOutput").ap()
    aw_out = nc.dram_tensor("aw_out", [BPC, L], F32, kind="ExternalOutput").ap()

    with tile.TileContext(nc) as tc, ExitStack() as ctx:
        p_misc = ctx.enter_context(tc.tile_pool(name="misc", bufs=1))
        p_nat = ctx.enter_context(tc.tile_pool(name="nat", bufs=BPC * KL))
        p_ft = ctx.enter_context(tc.tile_pool(name="ft", bufs=2 * KD))
        p_w1 = ctx.enter_context(tc.tile_pool(name="w1", bufs=KU))
        p_w2 = ctx.enter_context(tc.tile_pool(name="w2", bufs=KU))
        p_sc = ctx.enter_context(tc.tile_pool(name="sc", bufs=3))
        p_row = ctx.enter_context(tc.tile_pool(name="row", bufs=2))
        # PSUM: every tile rounds to a full bank; 2+2+2+2 = 8 banks.
        # pf=2 suffices: the deferred-logits matmul already gates PE on the
        # previous tanh, so PE never runs more than ~1 group ahead of ACT.
        pp_t = ctx.enter_context(tc.tile_pool(name="pp_t", bufs=2, space="PSUM"))
        pp_f = ctx.enter_context(tc.tile_pool(name="pp_f", bufs=2, space="PSUM"))
        pp_lg = ctx.enter_context(tc.tile_pool(name="pp_lg", bufs=2, space="PSUM"))
        pp_cx = ctx.enter_context(tc.tile_pool(name="pp_cx", bufs=2, space="PSUM"))

        ident = p_misc.tile([128, 128], F32R, name="ident")
        nc.gpsimd.memset(_f(ident[:, :]), 0.0)
        nc.gpsimd.affine_select(
            out=ident[:, :],
            in_=ident[:, :],
            compare_op=mybir.AluOpType.not_equal,
            fill=1.0,
            base=0,
            pattern=[[-1, 128]],
            channel_multiplier=1,
        )

        # ---- small aux loads (SWDGE: separate path from the big sync ring)
        hid_sb = p_misc.tile([BPC, H], F32R, name="hid_sb")
        nc.gpsimd.dma_start(hid_sb[:, :], _r(hid))
        b1 = p_misc.tile([128, KU], F32, name="b1")
        nc.gpsimd.dma_start(b1[:, :], w1b.rearrange("(m p) -> p m", p=128))
        b2 = p_misc.tile([128, KU], F32, name="b2")
        nc.gpsimd.dma_start(b2[:, :], w2b.rearrange("(m p) -> p m", p=128))
        vsb = p_misc.tile([128, KU], F32R, name="vsb")
        nc.gpsimd.dma_start(vsb[:, :], _r(vw.rearrange("(m p) o -> p m o", p=128)))

        bsum = p_misc.tile([128, KU], F32, name="bsum")
        nc.vector.tensor_add(bsum[:, :], b1[:, :], b2[:, :])

        # hiddenT: [128(h), BPC] per h-tile, via PE transpose of [8, 128] strips
        hT = p_misc.tile([128, KH * BPC], F32R, name="hT")
        for k in range(KH):
            tp = pp_t.tile([128, 2 * 128], F32R, name=f"tp_h{k}", tag="tp")
            nc.tensor.transpose(
                tp[:, 0:BPC],
                hid_sb[0:BPC, k * 128 : (k + 1) * 128],
                ident[0:BPC, 0:BPC],
            )
            nc.vector.tensor_copy(hT[:, k * BPC : (k + 1) * BPC], tp[:, 0:BPC])

        # ---- feature tiles (sync ring): nat0, nat1 first, then W1 column
        # blocks streamed, then the rest of the batches.
        nat = {}

        def load_nat(b):
            for j in range(KL):
                t = p_nat.tile([128, D], F32R, name=f"nat{b}_{j}", tag="nat")
                nc.sync.dma_start(t[:, :], _r(feat[b, j * 128 : (j + 1) * 128, :]))
                nat[(b, j)] = t

        load_nat(0)

        # W1 column blocks [D, 128] -> [128(d within k), (k, u)], interleaved
        # with the remaining feature loads so GEMM[0] starts early and then
        # follows the W1 stream.
        def load_w1m(m):
            t = p_w1.tile([128, D], F32R, name=f"w1m{m}", tag="w1")
            nc.sync.dma_start(
                t[:, :],
                _r(w1[:, m * 128 : (m + 1) * 128].rearrange("(k p) u -> p k u", p=128)),
            )
            return t

        w1m = [load_w1m(0), load_w1m(1)]
        load_nat(1)
        for m in range(2, KU):
            w1m.append(load_w1m(m))
        for b in range(2, BPC):
            load_nat(b)

        # ---- h_proj -> fused per-u-tile bias hbias[:, m*BPC + b] (W2 column
        # blocks on the SWDGE path, overlapping the sync ring)
        hbias = p_misc.tile([128, KU * BPC], F32, name="hbias")
        for m in range(KU):
            w2m = p_w2.tile([128, H], F32R, name=f"w2m{m}", tag="w2")
            nc.gpsimd.dma_start(
                w2m[:, :],
                _r(w2[:, m * 128 : (m + 1) * 128].rearrange("(k p) u -> p k u", p=128)),
            )
            ph = pp_f.tile([128, BPC], F32, name=f"ph{m}", tag="facc")
            for k in range(KH):
                nc.tensor.matmul(
                    ph[:, :],
                    w2m[:, k * 128 : (k + 1) * 128],
                    hT[:, k * BPC : (k + 1) * BPC],
                    start=(k == 0),
                    stop=(k == KH - 1),
                )
            nc.vector.tensor_scalar(
                out=hbias[:, m * BPC : (m + 1) * BPC],
                in0=ph[:, :],
                scalar1=bsum[:, m : m + 1],
                scalar2=None,
                op0=mybir.AluOpType.add,
            )

        # ---- pipelined phases over batch PAIRS (N=512 moving operand)
        featT = {}

        def do_T(p, ks=range(KD)):
            # featT2[(p,k)] = [128(d), 512]: cols 0-255 batch 2p, 256-511 2p+1
            for k in ks:
                ft = p_ft.tile([128, 2 * L], F32R, name=f"ft{p}_{k}", tag="ft")
                tp = pp_t.tile([128, 512], F32R, name=f"tp{p}_{k}", tag="tp")
                for half in range(2):
                    for j in range(KL):
                        nc.tensor.transpose(
                            tp[:, half * L + j * 128 : half * L + (j + 1) * 128],
                            nat[(2 * p + half, j)][:, k * 128 : (k + 1) * 128],
                            ident[:, :],
                        )
                nc.vector.tensor_copy(ft[:, :], tp[:, :])
                featT[(p, k)] = ft

        def do_softmax(b, lg, off):
            # softmax of one [1,256] half of the pair's logits row
            ex_row = p_row.tile([1, L], F32, name=f"ex{b}", tag="ex")
            ssum = p_row.tile([1, 1], F32, name=f"ssum{b}", tag="ssum")
            nc.scalar.activation(
                ex_row[0:1, :], lg[0:1, off : off + L], AF.Exp, accum_out=ssum[0:1, :]
            )
            rec = p_row.tile([1, 1], F32, name=f"rec{b}", tag="rec")
            nc.vector.reciprocal(rec[0:1, :], ssum[0:1, :])
            aw_row = p_row.tile([1, L], F32R, name=f"aw{b}", tag="aw")
            nc.vector.tensor_scalar_mul(aw_row[0:1, :], ex_row[0:1, :], rec[0:1, :])
            nc.scalar.dma_start(aw_out[b : b + 1, :], _f(aw_row[0:1, :]))
            # awT columns [128(l), j] via tiny partition-spread sb2sb DMAs on
            # the otherwise-idle SWDGE path
            awT_b = p_row.tile([128, KL], F32R, name=f"awT{b}", tag="awT")
            for j in range(KL):
                nc.gpsimd.dma_start(
                    awT_b[:, j : j + 1], aw_row[0:1, j * 128 : (j + 1) * 128]
                )
            return awT_b

        def do_ctx(b, awT_b):
            crow = p_row.tile([1, D], F32, name=f"crow{b}", tag="crow")
            for h2 in range(2):
                cps = pp_cx.tile([1, 512], F32, name=f"cps{b}_{h2}", tag="cps")
                for j in range(KL):
                    nc.tensor.matmul(
                        cps[:, :],
                        awT_b[:, j : j + 1],
                        nat[(b, j)][:, h2 * 512 : (h2 + 1) * 512],
                        start=(j == 0),
                        stop=(j == KL - 1),
                    )
                nc.vector.tensor_copy(crow[0:1, h2 * 512 : (h2 + 1) * 512], cps[0:1, :])
            nc.scalar.dma_start(ctx_out[b : b + 1, :], crow[0:1, :])

        rep_ctx = ExitStack()
        if pipeline_reps > 1:  # benchmarking only: repeat pipeline in a HW loop
            rep_ctx.enter_context(tc.For_i(0, pipeline_reps, 1))

        NP = BPC // 2  # batch pairs
        do_T(0)
        pend_ctx = []
        for p in range(NP):
            lg = pp_lg.tile([1, 2 * L], F32, name=f"lg{p}", tag="lg")
            pend = None
            for m in range(KU):
                fa = pp_f.tile([128, 2 * L], F32, name=f"fa{p}_{m}", tag="facc")
                for k in range(KD):
                    nc.tensor.matmul(
                        fa[:, :],
                        w1m[m][:, k * 128 : (k + 1) * 128],
                        featT[(p, k)][:, :],
                        start=(k == 0),
                        stop=(k == KD - 1),
                    )
                sc = p_sc.tile([128, 2 * L], F32R, name=f"sc{p}_{m}", tag="sc")
                for half in range(2):
                    b = 2 * p + half
                    nc.scalar.activation(
                        sc[:, half * L : (half + 1) * L],
                        fa[:, half * L : (half + 1) * L],
                        AF.Tanh,
                        bias=hbias[:, m * BPC + b : m * BPC + b + 1],
                        scale=1.0,
                    )
                # deferred logits matmul: keeps PE from stalling on the tanh
                if pend is not None:
                    mm, ss = pend
                    nc.tensor.matmul(
                        lg[:, :],
                        vsb[:, mm : mm + 1],
                        ss[:, :],
                        start=(mm == 0),
                        stop=False,
                        skip_group_check=True,
                    )
                pend = (m, sc)
                if m == 3 and pend_ctx:
                    do_ctx(*pend_ctx.pop(0))
                if m == 6 and pend_ctx:
                    do_ctx(*pend_ctx.pop(0))
            # next pair's transposes fill PE while the last tanh finishes;
            # softmax's DVE ops run between the two copy halves so the awT
            # chain completes early.
            if p + 1 < NP:
                do_T(p + 1, range(0, KD // 2))
            mm, ss = pend
            nc.tensor.matmul(
                lg[:, :],
                vsb[:, mm : mm + 1],
                ss[:, :],
                start=False,
                stop=True,
                skip_group_check=True,
            )
            pend_ctx.append((2 * p, do_softmax(2 * p, lg, 0)))
            pend_ctx.append((2 * p + 1, do_softmax(2 * p + 1, lg, L)))
            if p + 1 < NP:
                do_T(p + 1, range(KD // 2, KD))
        for args in pend_ctx:
            do_ctx(*args)
        rep_ctx.close()

    nc.compile()
    return nc


_CACHE = {}


def _get_nc():
    if "nc" not in _CACHE:
        _CACHE["nc"] = build_nc()
    return _CACHE["nc"]


def run(features, hidden, W1_w, W1_b, W2_w, W2_b, V_w, trace=False):
    nc = _get_nc()
    in_maps = []
    for c in range(NCORES):
        sl = slice(c * BPC, (c + 1) * BPC)
        in_maps.append(
            {
                "features": np.ascontiguousarray(features[sl], dtype=np.float32),
                "hidden": np.ascontiguousarray(hidden[sl], dtype=np.float32),
                "W1_w": np.ascontiguousarray(W1_w, dtype=np.float32),
                "W1_b": np.ascontiguousarray(W1_b, dtype=np.float32),
                "W2_w": np.ascontiguousarray(W2_w, dtype=np.float32),
                "W2_b": np.ascontiguousarray(W2_b, dtype=np.float32),
                "V_w": np.ascontiguousarray(V_w, dtype=np.float32),
            }
        )
    if trace:
        try:
            from antenv.axon_hooks import get_axon_ntff_profile_hook  # noqa: F401
        except ImportError:
            trace = False  # no NTFF hook in this container
    res = run_bass_kernel_spmd(nc, in_maps, core_ids=list(range(NCORES)), trace=trace)
    ctxv = np.concatenate([r["ctx_out"] for r in res.results], axis=0)
    aw = np.concatenate([r["aw_out"] for r in res.results], axis=0).reshape(B, L, 1)
    return (ctxv, aw), res


def kernel(features, hidden, W1_w, W1_b, W2_w, W2_b, V_w, V_b):
    # V_b shifts every logit equally and softmax is shift-invariant, so it
    # does not affect either output.
    del V_b
    trace = bool(int(os.environ.get("KERNEL_TRACE", "0")))
    (ctxv, aw), _res = run(
        np.asarray(features),
        np.asarray(hidden),
        np.asarray(W1_w),
        np.asarray(W1_b),
        np.asarray(W2_w),
        np.asarray(W2_b),
        np.asarray(V_w),
        trace=trace,
    )
    if trace and _res.exec_time_ns is not None:
        print(f"HW exec time: {_res.exec_time_ns} ns")
    return ctxv, aw


# revision 30
# speedup vs baseline: 1.0174x; 1.0174x over previous
# Bahdanau-style additive attention, fused single-pass Trainium2 kernel.
#
# Reference computation (per batch element b):
#   f_proj = features[b] @ W1_w + W1_b          [L, U]
#   h_proj = hidden[b] @ W2_w + W2_b            [U]
#   score  = tanh(f_proj + h_proj)              [L, U]
#   logits = score @ V_w (+ V_b)                [L]
#   aw     = softmax_L(logits)                  [L]   (V_b cancels in softmax)
#   ctx    = sum_l aw[l] * features[b, l, :]    [D]
#
# Sharding: data-parallel over batch B=64 across 8 NeuronCores (8 per core),
# weights replicated. Everything between the feature load and the two tiny
# outputs stays on-chip.
#
# Per-core structure (b = 0..7 local batches), fully software-pipelined:
#  - nat[b,j] = [128(l), 1024(d)] natural feature tiles, resident (reused by
#    the context matmul at the end of each batch).
#  - featT[b,k] = [128(d), 256(l)] via PE transposes + one DVE copy per tile.
#  - W1/W2 streamed as column blocks [H,128] -> [128,(k,u)] so u-tile m only
#    waits for its own 512KB, letting the GEMM start ~8us into the kernel.
#  - f_projT accumulated over 8 d-tiles in PSUM; ScalarE tanh(psum + bias)
#    with bias = (hidden@W2 + W1_b + W2_b)[u] per partition, batched upfront.
#  - logits row via matmul with V column stationary, accumulated over u-tiles.
#  - per-batch softmax on the [1,256] PSUM row (exp w/ fused row-sum on ACT),
#    aw transposed back to a [128,1] column, context row via matmul with the
#    aw column stationary against nat tiles; all overlapped with the next
#    batch's GEMM.
#
# Matmuls/transposes run as float32r (fp32 replication mode): measured
# ~1.3 cyc/row on HW vs 5 cyc/row for plain fp32, at ~2e-4 relative error.
# Walrus requires every f32r matmul operand to be *produced* as f32r, so the
# feeding tiles are declared float32r; non-matmul readers bitcast to f32.

import os
from contextlib import ExitStack

import numpy as np

import concourse.mybir as mybir
import concourse.tile as tile
from concourse import bacc
from concourse.bass_utils import run_bass_kernel_spmd

B, L, D, H, U = 64, 256, 1024, 1024, 1024
NCORES = 8
BPC = B // NCORES  # local batches per core
KL = L // 128      # 2 l-tiles
KD = D // 128      # 8 d-tiles
KU = U // 128      # 8 u-tiles
KH = H // 128      # 8 h-tiles

F32 = mybir.dt.float32
F32R = mybir.dt.float32r
AF = mybir.ActivationFunctionType


def _r(ap):
    return ap.bitcast(F32R)


def _f(ap):
    return ap.bitcast(F32)


def build_nc(pipeline_reps=1):
    nc = bacc.Bacc(target_bir_lowering=False, debug=False)

    feat = nc.dram_tensor("features", [BPC, L, D], F32, kind="ExternalInput").ap()
    hid = nc.dram_tensor("hidden", [BPC, H], F32, kind="ExternalInput").ap()
    w1 = nc.dram_tensor("W1_w", [D, U], F32, kind="ExternalInput").ap()
    w1b = nc.dram_tensor("W1_b", [U], F32, kind="ExternalInput").ap()
    w2 = nc.dram_tensor("W2_w", [H, U], F32, kind="ExternalInput").ap()
    w2b = nc.dram_tensor("W2_b", [U], F32, kind="ExternalInput").ap()
    vw = nc.dram_tensor("V_w", [U, 1], F32, kind="ExternalInput").ap()
    ctx_out = nc.dram_tensor("ctx_out", [BPC, D], F32, kind="ExternalOutput").ap()
    aw_out = nc.dram_tensor("aw_out", [BPC, L], F32, kind="ExternalOutput").ap()

    with tile.TileContext(nc) as tc, ExitStack() as ctx:
        p_misc = ctx.enter_context(tc.tile_pool(name="misc", bufs=1))
        p_nat = ctx.enter_context(tc.tile_pool(name="nat", bufs=BPC * KL))
        p_ft = ctx.enter_context(tc.tile_pool(name="ft", bufs=2 * KD))
        p_w1 = ctx.enter_context(tc.tile_pool(name="w1", bufs=KU))
        p_w2 = ctx.enter_context(tc.tile_pool(name="w2", bufs=KU))
        p_sc = ctx.enter_context(tc.tile_pool(name="sc", bufs=3))
        p_row = ctx.enter_context(tc.tile_pool(name="row", bufs=2))
        # PSUM: every tile rounds to a full bank; 2+2+2+2 = 8 banks.
        # pf=2 suffices: the deferred-logits matmul already gates PE on the
        # previous tanh, so PE never runs more than ~1 group ahead of ACT.
        pp_t = ctx.enter_context(tc.tile_pool(name="pp_t", bufs=2, space="PSUM"))
        pp_f = ctx.enter_context(tc.tile_pool(name="pp_f", bufs=2, space="PSUM"))
        pp_lg = ctx.enter_context(tc.tile_pool(name="pp_lg", bufs=2, space="PSUM"))
        pp_cx = ctx.enter_context(tc.tile_pool(name="pp_cx", bufs=2, space="PSUM"))

        ident = p_misc.tile([128, 128], F32R, name="ident")
        nc.gpsimd.memset(_f(ident[:, :]), 0.0)
        nc.gpsimd.affine_select(
            out=ident[:, :],
            in_=ident[:, :],
            compare_op=mybir.AluOpType.not_equal,
            fill=1.0,
            base=0,
            pattern=[[-1, 128]],
            channel_multiplier=1,
        )

        # ---- small aux loads (SWDGE: separate path from the big sync ring)
        hid_sb = p_misc.tile([BPC, H], F32R, name="hid_sb")
        nc.gpsimd.dma_start(hid_sb[:, :], _r(hid))
        b1 = p_misc.tile([128, KU], F32, name="b1")
        nc.gpsimd.dma_start(b1[:, :], w1b.rearrange("(m p) -> p m", p=128))
        b2 = p_misc.tile([128, KU], F32, name="b2")
        nc.gpsimd.dma_start(b2[:, :], w2b.rearrange("(m p) -> p m", p=128))
        vsb = p_misc.tile([128, KU], F32R, name="vsb")
        nc.gpsimd.dma_start(vsb[:, :], _r(vw.rearrange("(m p) o -> p m o", p=128)))

        bsum = p_misc.tile([128, KU], F32, name="bsum")
        nc.vector.tensor_add(bsum[:, :], b1[:, :], b2[:, :])

        # hiddenT: [128(h), BPC] per h-tile, via PE transpose of [8, 128] strips
        hT = p_misc.tile([128, KH * BPC], F32R, name="hT")
        for k in range(KH):
            tp = pp_t.tile([128, 2 * 128], F32R, name=f"tp_h{k}", tag="tp")
            nc.tensor.transpose(
                tp[:, 0:BPC],
                hid_sb[0:BPC, k * 128 : (k + 1) * 128],
                ident[0:BPC, 0:BPC],
            )
            nc.vector.tensor_copy(hT[:, k * BPC : (k + 1) * BPC], tp[:, 0:BPC])

        # ---- feature tiles (sync ring): nat0, nat1 first, then W1 column
        # blocks streamed, then the rest of the batches.
        nat = {}

        def load_nat(b):
            for j in range(KL):
                t = p_nat.tile([128, D], F32R, name=f"nat{b}_{j}", tag="nat")
                nc.sync.dma_start(t[:, :], _r(feat[b, j * 128 : (j + 1) * 128, :]))
                nat[(b, j)] = t

        load_nat(0)

        # W1 column blocks [D, 128] -> [128(d within k), (k, u)], interleaved
        # with the remaining feature loads so GEMM[0] starts early and then
        # follows the W1 stream.
        def load_w1m(m):
            t = p_w1.tile([128, D], F32R, name=f"w1m{m}", tag="w1")
            nc.sync.dma_start(
                t[:, :],
                _r(w1[:, m * 128 : (m + 1) * 128].rearrange("(k p) u -> p k u", p=128)),
            )
            return t

        load_nat(1)  # pair 0 = batches 0+1: both needed before the first T
        w1m = [load_w1m(m) for m in range(KU)]
        for b in range(2, BPC):
            load_nat(b)

        # ---- h_proj -> fused per-u-tile bias hbias[:, m*BPC + b] (W2 column
        # blocks on the SWDGE path, overlapping the sync ring)
        hbias = p_misc.tile([128, KU * BPC], F32, name="hbias")
        for m in range(KU):
            w2m = p_w2.tile([128, H], F32R, name=f"w2m{m}", tag="w2")
            nc.gpsimd.dma_start(
                w2m[:, :],
                _r(w2[:, m * 128 : (m + 1) * 128].rearrange("(k p) u -> p k u", p=128)),
            )
            ph = pp_f.tile([128, BPC], F32, name=f"ph{m}", tag="facc")
            for k in range(KH):
                nc.tensor.matmul(
                    ph[:, :],
                    w2m[:, k * 128 : (k + 1) * 128],
                    hT[:, k * BPC : (k + 1) * BPC],
                    start=(k == 0),
                    stop=(k == KH - 1),
                )
            nc.vector.tensor_scalar(
                out=hbias[:, m * BPC : (m + 1) * BPC],
                in0=ph[:, :],
                scalar1=bsum[:, m : m + 1],
                scalar2=None,
                op0=mybir.AluOpType.add,
            )

        # ---- pipelined phases over batch PAIRS (N=512 moving operand)
        featT = {}

        def do_T(p, ks=range(KD)):
            # featT2[(p,k)] = [128(d), 512]: cols 0-255 batch 2p, 256-511 2p+1
            for k in ks:
                ft = p_ft.tile([128, 2 * L], F32R, name=f"ft{p}_{k}", tag="ft")
                tp = pp_t.tile([128, 512], F32R, name=f"tp{p}_{k}", tag="tp")
                for half in range(2):
                    for j in range(KL):
                        nc.tensor.transpose(
                            tp[:, half * L + j * 128 : half * L + (j + 1) * 128],
                            nat[(2 * p + half, j)][:, k * 128 : (k + 1) * 128],
                            ident[:, :],
                        )
                nc.vector.tensor_copy(ft[:, :], tp[:, :])
                featT[(p, k)] = ft

        def do_softmax(b, lg, off):
            # softmax of one [1,256] half of the pair's logits row
            ex_row = p_row.tile([1, L], F32, name=f"ex{b}", tag="ex")
            ssum = p_row.tile([1, 1], F32, name=f"ssum{b}", tag="ssum")
            nc.scalar.activation(
                ex_row[0:1, :], lg[0:1, off : off + L], AF.Exp, accum_out=ssum[0:1, :]
            )
            rec = p_row.tile([1, 1], F32, name=f"rec{b}", tag="rec")
            nc.vector.reciprocal(rec[0:1, :], ssum[0:1, :])
            aw_row = p_row.tile([1, L], F32R, name=f"aw{b}", tag="aw")
            nc.vector.tensor_scalar_mul(aw_row[0:1, :], ex_row[0:1, :], rec[0:1, :])
            nc.scalar.dma_start(aw_out[b : b + 1, :], _f(aw_row[0:1, :]))
            # awT columns [128(l), j] via tiny partition-spread sb2sb DMAs on
            # the otherwise-idle SWDGE path
            awT_b = p_row.tile([128, KL], F32R, name=f"awT{b}", tag="awT")
            for j in range(KL):
                nc.gpsimd.dma_start(
                    awT_b[:, j : j + 1], aw_row[0:1, j * 128 : (j + 1) * 128]
                )
            return awT_b

        def do_ctx(b, awT_b):
            crow = p_row.tile([1, D], F32, name=f"crow{b}", tag="crow")
            for h2 in range(2):
                cps = pp_cx.tile([1, 512], F32, name=f"cps{b}_{h2}", tag="cps")
                for j in range(KL):
                    nc.tensor.matmul(
                        cps[:, :],
                        awT_b[:, j : j + 1],
                        nat[(b, j)][:, h2 * 512 : (h2 + 1) * 512],
                        start=(j == 0),
                        stop=(j == KL - 1),
                    )
                nc.vector.tensor_copy(crow[0:1, h2 * 512 : (h2 + 1) * 512], cps[0:1, :])
            nc.scalar.dma_start(ctx_out[b : b + 1, :], crow[0:1, :])

        rep_ctx = ExitStack()
        if pipeline_reps > 1:  # benchmarking only: repeat pipeline in a HW loop
            rep_ctx.enter_context(tc.For_i(0, pipeline_reps, 1))

        NP = BPC // 2  # batch pairs
        do_T(0)
        pend_ctx = []
        for p in range(NP):
            lg = pp_lg.tile([1, 2 * L], F32, name=f"lg{p}", tag="lg")
            pend = None
            for m in range(KU):
                fa = pp_f.tile([128, 2 * L], F32, name=f"fa{p}_{m}", tag="facc")
                for k in range(KD):
                    nc.tensor.matmul(
                        fa[:, :],
                        w1m[m][:, k * 128 : (k + 1) * 128],
                        featT[(p, k)][:, :],
                        start=(k == 0),
                        stop=(k == KD - 1),
                    )
                sc = p_sc.tile([128, 2 * L], F32R, name=f"sc{p}_{m}", tag="sc")
                for half in range(2):
                    b = 2 * p + half
                    nc.scalar.activation(
                        sc[:, half * L : (half + 1) * L],
                        fa[:, half * L : (half + 1) * L],
                        AF.Tanh,
                        bias=hbias[:, m * BPC + b : m * BPC + b + 1],
                        scale=1.0,
                    )
                # deferred logits matmul: keeps PE from stalling on the tanh
                if pend is not None:
                    mm, ss = pend
                    nc.tensor.matmul(
                        lg[:, :],
                        vsb[:, mm : mm + 1],
                        ss[:, :],
                        start=(mm == 0),
                        stop=False,
                        skip_group_check=True,
                    )
                pend = (m, sc)
                # interleave ONE transpose group of the next pair per m-group:
                # PE emits 4 transposes (~0.4us) into one PSUM tile and DVE has
                # a full m-group (~2.3us) to drain it — no PSUM-slot stall,
                # unlike emitting all 32 boundary transposes in a clump.
                if p + 1 < NP:
                    do_T(p + 1, [m])
                if m == 3 and pend_ctx:
                    do_ctx(*pend_ctx.pop(0))
                if m == 6 and pend_ctx:
                    do_ctx(*pend_ctx.pop(0))
            mm, ss = pend
            nc.tensor.matmul(
                lg[:, :],
                vsb[:, mm : mm + 1],
                ss[:, :],
                start=False,
                stop=True,
                skip_group_check=True,
            )
            pend_ctx.append((2 * p, do_softmax(2 * p, lg, 0)))
            pend_ctx.append((2 * p + 1, do_softmax(2 * p + 1, lg, L)))
        for args in pend_ctx:
            do_ctx(*args)
        rep_ctx.close()

    nc.compile()
    return nc


_CACHE = {}


def _get_nc():
    if "nc" not in _CACHE:
        _CACHE["nc"] = build_nc()
    return _CACHE["nc"]


def run(features, hidden, W1_w, W1_b, W2_w, W2_b, V_w, trace=False):
    nc = _get_nc()
    in_maps = []
    for c in range(NCORES):
        sl = slice(c * BPC, (c + 1) * BPC)
        in_maps.append(
            {
                "features": np.ascontiguousarray(features[sl], dtype=np.float32),
                "hidden": np.ascontiguousarray(hidden[sl], dtype=np.float32),
                "W1_w": np.ascontiguousarray(W1_w, dtype=np.float32),
                "W1_b": np.ascontiguousarray(W1_b, dtype=np.float32),
                "W2_w": np.ascontiguousarray(W2_w, dtype=np.float32),
                "W2_b": np.ascontiguousarray(W2_b, dtype=np.float32),
                "V_w": np.ascontiguousarray(V_w, dtype=np.float32),
            }
        )
    if trace:
        try:
            from antenv.axon_hooks import get_axon_ntff_profile_hook  # noqa: F401
        except ImportError:
            trace = False  # no NTFF hook in this container
    res = run_bass_kernel_spmd(nc, in_maps, core_ids=list(range(NCORES)), trace=trace)
    ctxv = np.concatenate([r["ctx_out"] for r in res.results], axis=0)
    aw = np.concatenate([r["aw_out"] for r in res.results], axis=0).reshape(B, L, 1)
    return (ctxv, aw), res


def kernel(features, hidden, W1_w, W1_b, W2_w, W2_b, V_w, V_b):
    # V_b shifts every logit equally and softmax is shift-invariant, so it
    # does not affect either output.
    del V_b
    trace = bool(int(os.environ.get("KERNEL_TRACE", "0")))
    (ctxv, aw), _res = run(
        np.asarray(features),
        np.asarray(hidden),
        np.asarray(W1_w),
        np.asarray(W1_b),
        np.asarray(W2_w),
        np.asarray(W2_b),
        np.asarray(V_w),
        trace=trace,
    )
    if trace and _res.exec_time_ns is not None:
        print(f"HW exec time: {_res.exec_time_ns} ns")
    return ctxv, aw
